# revision 19
# baseline (speedup 1.0000x reference)
"""Trainium2 Bass kernel for nn_Memory scatter_memory problem — v2.

Reference computation:
    scale = t/(t+1) if t > 1 else 1 ; inv = 1/(t+1)
    entity_memory = entity_memory*scale ; .at[nodes_ids].add((nodes_emb @ W_node.T + b_node)*inv)
    rel_memory    = rel_memory*scale    ; .at[rels_ids].add((rels_emb @ W_rel.T + b_rel)*inv)
    out = concat([entity_memory, rel_memory])   # [100500, 512]

v2 strategy (8 NeuronCores, SPMD):
  - Row-shard both tables; host routes events to owner core, sorts node
    events by local row id (baseline approach).
  - NODES: fp8(e4m3) DoubleRow projection (2 k-tiles per matmul, half
    cycles/col), updates quantized to fp8, scatter-add via fp8 DoubleRow
    one-hot matmuls over chunk PAIRS (2 chunks per scatter matmul).
  - RELS: aggregate raw embeddings by rel id first (one-hot matmul over
    events, fp16), then ONE tiny projection of the 64 aggregated rows.
    Removes the entire per-event rel projection.
  - Memory table streamed bf16 in, out written bf16 (nodes) / f32 (rels).
  - Elementwise spread across DVE / Pool / Act engines; batched DMAs.
"""

import os
import sys
import numpy as np

for _p in ("/root/.axon_site", "/root/.axon_site/_ro/trn_rl_repo",
           "/root/.axon_site/_ro/pypackages", "/opt/trn_rl_repo"):
    if os.path.isdir(_p) and _p not in sys.path:
        sys.path.append(_p)

import ml_dtypes
import concourse.bacc as bacc
import concourse.mybir as mybir
import concourse.tile as tile
from concourse.bass_utils import run_bass_kernel_spmd

F32 = mybir.dt.float32
F16 = mybir.dt.float16
BF16 = mybir.dt.bfloat16
F8 = mybir.dt.float8e4
AL = mybir.AluOpType
DR = mybir.MatmulPerfMode.DoubleRow

NP_F8 = ml_dtypes.float8_e4m3
NP_BF16 = ml_dtypes.bfloat16

N_NODES = 100000
N_RELS = 500
MEM_DIM = 512
IN_DIM = 1024
NCORES = 8
NSHARD = 12544          # 98 * 128 rows per core (core 7 ragged, padded)
NGROUPS = NSHARD // 128  # 98
RSHARD = 64             # rel rows per core (core 7 ragged, padded)
KT = IN_DIM // 128      # 8 k-tiles
KP = KT // 2            # 4 DoubleRow k-pairs
PAD_ID = 1.0e6
W_SCALE = 128.0         # fp8 scaling of W_node*inv (descale at merge)
GB = 8                  # groups per mem/out DMA batch
NB = (NGROUPS + GB - 1) // GB
MEM_BF16 = bool(int(os.environ.get("KERNEL_MEM_BF16", "0")))
MEM_DT = BF16 if MEM_BF16 else F8
MEM_NP = NP_BF16 if MEM_BF16 else NP_F8

_module_cache = {}


def _ensure_ntff_hook():
    """Register the axon NTFF profile hook (missing antenv.axon_hooks shim)."""
    import types
    try:
        from antenv.axon_hooks import get_axon_ntff_profile_hook
        return get_axon_ntff_profile_hook() is not None
    except ImportError:
        pass
    try:
        import antenv
        from trn_agent_boot.trn_boot import _ntff_profile_via_ctypes
        import concourse.bass_utils as bu
        mod = types.ModuleType("antenv.axon_hooks")
        state = {"h": None}
        mod.set_axon_ntff_profile_hook = lambda h: state.__setitem__("h", h)
        mod.get_axon_ntff_profile_hook = lambda: state["h"]
        sys.modules["antenv.axon_hooks"] = mod
        antenv.axon_hooks = mod
        h = _ntff_profile_via_ctypes("/opt/axon/libaxon_pjrt.so")
        mod.set_axon_ntff_profile_hook(h)
        bu.upload_artifacts = lambda tmpdir: f"local:{tmpdir}"
        return h is not None
    except Exception:
        return False


def _build_module(NCn, NCr, pair_spans, has_bias):
    """Build the SPMD Bacc module.

    NCn: node event chunks (even; last may be all-pad). NCr: rel chunks.
    pair_spans: tuple over pair j of tuple of groups touched by chunk 2j
    or 2j+1 (union, sorted).
    """
    nc = bacc.Bacc(None, target_bir_lowering=False)

    NPn = NCn // 2
    NPr = (NCr + 1) // 2
    # widest contiguous group range any pair spans (for one-hot tiles)
    NS_MAX = max((s[-1] - s[0] + 1) for s in pair_spans if s)
    IW = max(NS_MAX * 128, 128)

    emb_n = nc.dram_tensor("emb_n", [NPn, 128, 2 * KT * 128], F8, kind="ExternalInput")
    emb_r = nc.dram_tensor("emb_r", [NPr, 128, 2 * IN_DIM], F16, kind="ExternalInput")
    ids_n = nc.dram_tensor("ids_n", [128, NCn], F32, kind="ExternalInput")
    ids_r = nc.dram_tensor("ids_r", [128, NCr], F32, kind="ExternalInput")
    w_n = nc.dram_tensor("w_n", [128, KP * 2 * MEM_DIM], F8, kind="ExternalInput")
    w_r = nc.dram_tensor("w_r", [128, KT * MEM_DIM], F16, kind="ExternalInput")
    b_n = nc.dram_tensor("b_n", [128, MEM_DIM], F32, kind="ExternalInput")
    iota_in = nc.dram_tensor("iota_in", [128, IW], F32, kind="ExternalInput")
    ident_in = nc.dram_tensor("ident_in", [128, 128], F16, kind="ExternalInput")
    rmem_s = nc.dram_tensor("rmem_s", [RSHARD, MEM_DIM], F32, kind="ExternalInput")
    mem_t = nc.dram_tensor("mem_t", [128, NGROUPS * MEM_DIM], MEM_DT, kind="ExternalInput")
    out_n = nc.dram_tensor("out_n", [128, NGROUPS * MEM_DIM], BF16, kind="ExternalOutput")
    out_r = nc.dram_tensor("out_r", [RSHARD, MEM_DIM], F32, kind="ExternalOutput")

    # group lifetime over pairs
    first_pair, last_pair = {}, {}
    for j, gs in enumerate(pair_spans):
        for g in gs:
            first_pair.setdefault(g, j)
            last_pair[g] = j
    # PSUM budget
    maxopen, open_now = 0, set()
    for j, gs in enumerate(pair_spans):
        open_now.update(gs)
        maxopen = max(maxopen, len(open_now))
        for g in list(open_now):
            if last_pair[g] == j:
                open_now.discard(g)
    pg_bufs = min(max(maxopen, 1), 5)
    pu_bufs = 8 - 2 - pg_bufs  # agg pool has 2 banks
    assert pu_bufs >= 1, f"PSUM overflow: maxopen={maxopen}"

    with tile.TileContext(nc) as tc:
        with tc.tile_pool(name="const", bufs=1) as cpool, \
             tc.tile_pool(name="en", bufs=8) as enpool, \
             tc.tile_pool(name="er", bufs=8) as erpool, \
             tc.tile_pool(name="oh", bufs=4) as ohpool, \
             tc.tile_pool(name="ohr", bufs=6) as ohrpool, \
             tc.tile_pool(name="upd", bufs=4) as updpool, \
             tc.tile_pool(name="mem", bufs=4) as mpool, \
             tc.tile_pool(name="outs", bufs=4) as opool, \
             tc.tile_pool(name="tail", bufs=2) as tpool, \
             tc.tile_pool(name="pu", bufs=pu_bufs, space="PSUM") as pupool, \
             tc.tile_pool(name="pg", bufs=pg_bufs, space="PSUM") as pgpool, \
             tc.tile_pool(name="pa", bufs=2, space="PSUM") as papool:

            # ---- startup-critical constants only (tail consts deferred) ----
            t_wn = cpool.tile([128, KP, 2, MEM_DIM], F8, tag="wn")
            nc.sync.dma_start(t_wn[:], w_n.ap().rearrange("p (k i n) -> p k i n", k=KP, i=2))
            t_iota = cpool.tile([128, IW], F32, tag="iota")
            nc.scalar.dma_start(t_iota[:], iota_in[:])
            t_ids_n = cpool.tile([128, NCn], F32, tag="idsn")
            nc.scalar.dma_start(t_ids_n[:], ids_n[:])
            t_ids_r = cpool.tile([128, NCr], F32, tag="idsr")
            nc.scalar.dma_start(t_ids_r[:], ids_r[:])
            if has_bias:
                t_bn = cpool.tile([128, MEM_DIM], F32, tag="bn")
                nc.scalar.dma_start(t_bn[:], b_n[:])
            tail_consts = {}

            def fetch_tail_consts():
                t_wr = cpool.tile([128, KT, MEM_DIM], F16, tag="wr")
                nc.scalar.dma_start(t_wr[:], w_r.ap().rearrange("p (k n) -> p k n", k=KT))
                t_ident = cpool.tile([128, 128], F16, tag="ident")
                nc.scalar.dma_start(t_ident[:], ident_in[:])
                t_rmem = cpool.tile([RSHARD, MEM_DIM], F32, tag="rmem")
                nc.scalar.dma_start(t_rmem[:], rmem_s[:])
                tail_consts.update(wr=t_wr, ident=t_ident, rmem=t_rmem)

            # ---- mem-in batches (prefetched ahead of merge frontier) ----
            mem_tiles = {}

            def fetch_mem(b):
                if b in mem_tiles or b >= NB:
                    return
                gcnt = min(GB, NGROUPS - b * GB)
                t = mpool.tile([128, GB, MEM_DIM], MEM_DT, tag="mem", name=f"mem_{b}")
                nc.gpsimd.dma_start(
                    t[:, :gcnt, :],
                    mem_t.ap()[:, b * GB * MEM_DIM:(b * GB + gcnt) * MEM_DIM]
                    .rearrange("p (g n) -> p g n", g=gcnt))
                mem_tiles[b] = t

            for _b in range(3):
                fetch_mem(_b)

            # ---- rel aggregation PSUM (accumulates across all rel chunks) ----
            p_agg_a = papool.tile([RSHARD, MEM_DIM], F32, tag="pa", name="agg_a")
            p_agg_b = papool.tile([RSHARD, MEM_DIM], F32, tag="pa", name="agg_b")

            er_pair = {}

            def rel_chunk(ec):
                if ec % 2 == 0:
                    t_pe = erpool.tile([128, 2, IN_DIM], F16, tag="er", name=f"er_{ec}")
                    nc.sync.dma_start(
                        t_pe[:], emb_r[ec // 2].rearrange("p (c x) -> p c x", c=2))
                    er_pair[ec // 2] = t_pe
                t_er = er_pair[ec // 2][:, ec % 2]
                t_ohr = ohrpool.tile([128, RSHARD], F16, tag="ohr", name=f"ohr_{ec}")
                nc.vector.tensor_scalar(
                    t_ohr[:], t_iota[:, :RSHARD], 0.0, t_ids_r[:, ec:ec + 1],
                    op0=AL.add, op1=AL.is_equal)
                st = (ec == 0)
                sp = (ec == NCr - 1)
                nc.tensor.matmul(p_agg_a[:], t_ohr[:], t_er[:, 0:MEM_DIM],
                                 start=st, stop=sp, skip_group_check=True)
                nc.tensor.matmul(p_agg_b[:], t_ohr[:], t_er[:, MEM_DIM:IN_DIM],
                                 start=st, stop=sp, skip_group_check=True)

            # ---- node pair machinery ----
            en_pair = {}
            pair_state = {}
            grp_psum = {}

            upd_tiles = {}

            def node_pair_dma(j):
                t_pp = enpool.tile([128, 2, KP, 2, 128], F8, tag="en", name=f"en_{j}")
                nc.sync.dma_start(
                    t_pp[:], emb_n[j].rearrange("p (c k i e) -> p c k i e",
                                                c=2, k=KP, i=2))
                en_pair[j] = t_pp
                upd_tiles[j] = updpool.tile([128, 2, MEM_DIM], F8, tag="upd",
                                            name=f"upd_{j}")

            def node_chunk_proj(j, i):
                t_pp, t_up = en_pair[j], upd_tiles[j]
                p_u = pupool.tile([128, MEM_DIM], F32, tag="pu", name=f"pu_{j}_{i}")
                for k in range(KP):
                    nc.tensor.matmul(p_u[:], t_pp[:, i, k], t_wn[:, k],
                                     start=(k == 0), stop=(k == KP - 1),
                                     perf_mode=DR)
                if has_bias:
                    nc.vector.scalar_tensor_tensor(
                        t_up[:, i, :], p_u[:], 1.0, t_bn[:],
                        op0=AL.mult, op1=AL.add)
                else:
                    nc.scalar.activation(
                        t_up[:, i, :], p_u[:],
                        mybir.ActivationFunctionType.Copy)

            def node_pair_ohs(j):
                # one wide is_equal per chunk half covering the pair's whole
                # contiguous group range; scatter matmuls slice per group
                gs = pair_spans[j]
                t_oh = None
                if gs:
                    g0 = gs[0]
                    ns = gs[-1] - g0 + 1
                    t_oh = ohpool.tile([128, 2, NS_MAX * 128], F8, tag="oh",
                                       name=f"oh_{j}")
                    for i in (0, 1):
                        nc.vector.tensor_scalar(
                            t_oh[:, i, :ns * 128], t_iota[:, :ns * 128],
                            float(g0 * 128),
                            t_ids_n[:, 2 * j + i:2 * j + i + 1],
                            op0=AL.add, op1=AL.is_equal)
                pair_state[j] = (upd_tiles.pop(j), t_oh)

            def node_pair_scatter(j):
                t_up, t_oh = pair_state.pop(j)
                gs = pair_spans[j]
                if not gs:
                    return
                g0 = gs[0]
                for g in gs:
                    if g not in grp_psum:
                        grp_psum[g] = pgpool.tile([128, MEM_DIM], F32, tag="pg",
                                                  name=f"pg_{g}")
                    off = (g - g0) * 128
                    nc.tensor.matmul(grp_psum[g][:],
                                     t_oh[:, :, off:off + 128], t_up[:],
                                     start=(first_pair[g] == j),
                                     stop=(last_pair[g] == j),
                                     perf_mode=DR, skip_group_check=True)

            # ---- merge frontier (strict ascending group order) ----
            out_state = {"b": -1, "tile": None}
            frontier = {"g": 0}

            def flush_out():
                b = out_state["b"]
                if b < 0:
                    return
                gcnt = min(GB, NGROUPS - b * GB)
                nc.gpsimd.dma_start(
                    out_n.ap()[:, b * GB * MEM_DIM:(b * GB + gcnt) * MEM_DIM]
                    .rearrange("p (g n) -> p g n", g=gcnt),
                    out_state["tile"][:, :gcnt, :])
                out_state["b"] = -1
                out_state["tile"] = None

            def merge_group(g):
                b, s = g // GB, g % GB
                if out_state["b"] != b:
                    flush_out()
                    out_state["b"] = b
                    out_state["tile"] = opool.tile([128, GB, MEM_DIM], BF16,
                                                   tag="outs", name=f"outs_{b}")
                    fetch_mem(b + 3)
                t_mem = mem_tiles[b]
                t_out = out_state["tile"]
                if g in grp_psum:
                    eng = nc.vector
                    eng.scalar_tensor_tensor(
                        t_out[:, s, :], grp_psum[g][:], 1.0 / W_SCALE,
                        t_mem[:, s, :], op0=AL.mult, op1=AL.add)
                    del grp_psum[g]
                else:
                    nc.scalar.activation(t_out[:, s, :], t_mem[:, s, :],
                                         mybir.ActivationFunctionType.Copy)

            def merge_upto(jdone):
                g = frontier["g"]
                while g < NGROUPS and last_pair.get(g, -1) <= jdone:
                    merge_group(g)
                    g += 1
                frontier["g"] = g

            # ---- rel tail: copy agg out, transpose, project, merge ----
            # (emitted right after the last rel chunk so it overlaps the
            # remaining node pairs)
            def rel_tail():
                t_wr, t_ident, t_rmem = (tail_consts["wr"], tail_consts["ident"],
                                         tail_consts["rmem"])
                t_agg = tpool.tile([RSHARD, IN_DIM], F16, tag="agg")
                nc.scalar.activation(t_agg[:, 0:MEM_DIM], p_agg_a[:],
                                     mybir.ActivationFunctionType.Copy)
                nc.scalar.activation(t_agg[:, MEM_DIM:IN_DIM], p_agg_b[:],
                                     mybir.ActivationFunctionType.Copy)
                p_tr = papool.tile([128, KT, RSHARD], F16, tag="pa", name="p_tr")
                for k in range(KT):
                    nc.tensor.transpose(p_tr[:, k, :],
                                        t_agg[:, k * 128:(k + 1) * 128],
                                        t_ident[:RSHARD, :RSHARD])
                t_aggT = tpool.tile([128, KT, RSHARD], F16, tag="aggT")
                nc.scalar.activation(t_aggT[:], p_tr[:],
                                     mybir.ActivationFunctionType.Copy)
                p_rel = papool.tile([RSHARD, MEM_DIM], F32, tag="pa", name="p_rel")
                for k in range(KT):
                    nc.tensor.matmul(p_rel[:], t_aggT[:, k, :], t_wr[:, k, :],
                                     start=(k == 0), stop=(k == KT - 1))
                t_outr = tpool.tile([RSHARD, MEM_DIM], F32, tag="outr")
                nc.vector.tensor_tensor(t_outr[:], p_rel[:], t_rmem[:], op=AL.add)
                nc.sync.dma_start(out_r[:], t_outr[:])

            # ---- main pipeline (rels paced to finish slightly early) ----
            LAG = 1
            next_r = 0
            pace = NCr / max(NPn - 4, 1)
            tc_at = max(NPn - 10, 0)
            for j in range(NPn):
                if j == tc_at:
                    fetch_tail_consts()
                node_pair_dma(j)
                node_chunk_proj(j, 0)
                # a rel-agg matmul between the two projections gives the PE
                # ready work while chunk 0's psum->fp8 copy frees the buffer
                if next_r < NCr and next_r < (j + 1) * pace:
                    rel_chunk(next_r)
                    next_r += 1
                    if next_r == NCr:
                        rel_tail()
                node_chunk_proj(j, 1)
                node_pair_ohs(j)
                while next_r < NCr and next_r < (j + 1) * pace:
                    rel_chunk(next_r)
                    next_r += 1
                    if next_r == NCr:
                        rel_tail()
                if j >= LAG:
                    node_pair_scatter(j - LAG)
                    merge_upto(j - LAG)
            for j in range(max(NPn - LAG, 0), NPn):
                node_pair_scatter(j)
                merge_upto(j)
            while next_r < NCr:
                rel_chunk(next_r)
                next_r += 1
                if next_r == NCr:
                    rel_tail()
            merge_upto(NPn + 1)
            frontier_g = frontier["g"]
            for g in range(frontier_g, NGROUPS):
                merge_group(g)
            flush_out()

    nc.finalize()
    return nc


# ---------------- host-side routing / packing ----------------

def _route(ids, n_rows_per_core):
    owner = np.minimum(ids // n_rows_per_core, NCORES - 1)
    perms = []
    for c in range(NCORES):
        ev = np.nonzero(owner == c)[0]
        perms.append(ev)
    nmax = max(len(p) for p in perms)
    NC = (nmax + 127) // 128
    return perms, max(NC, 1)


def _pack_ids(local_ids, NC):
    n = len(local_ids)
    out = np.full(NC * 128, PAD_ID, dtype=np.float32)
    out[:n] = local_ids.astype(np.float32)
    return np.ascontiguousarray(out.reshape(NC, 128).T)  # [128, NC]


def _chunk_groups(loc_sorted, NC):
    """Exact per-chunk touched-group sets."""
    spans = []
    for ec in range(NC):
        seg = loc_sorted[ec * 128:(ec + 1) * 128]
        spans.append(sorted(set(int(v) // 128 for v in seg)))
    return spans


def _pair_spans(chunk_spans):
    NP_ = len(chunk_spans) // 2
    out = []
    for j in range(NP_):
        s = sorted(set(chunk_spans[2 * j]) | set(chunk_spans[2 * j + 1]))
        out.append(tuple(s))
    return tuple(out)


def _pack_emb_n(emb, perm, NCn):
    """emb [B, IN_DIM] f32 -> [NPn, 128, 2*KT*128] fp8 DoubleRow layout."""
    n = len(perm)
    C = NCn * 128
    g = np.zeros((C, IN_DIM), dtype=NP_F8)
    g[:n] = emb[perm].astype(NP_F8)
    # [NCn,128e,KP,2,128p] -> [NCn,128p,KP,2,128e]
    g = g.reshape(NCn, 128, KP, 2, 128).transpose(0, 4, 2, 3, 1)
    # pair chunks: [NPn,2,128,KP,2,128] -> [NPn,128,2,KP,2,128]
    g = g.reshape(NCn // 2, 2, 128, KP, 2, 128).transpose(0, 2, 1, 3, 4, 5)
    return np.ascontiguousarray(g.reshape(NCn // 2, 128, 2 * KT * 128))


def _pack_emb_r(emb, perm, NCr):
    """emb [B, IN_DIM] f32 -> [NPr, 128, 2*IN_DIM] fp16 event-major pairs."""
    n = len(perm)
    NPr = (NCr + 1) // 2
    C = NPr * 256
    g = np.zeros((C, IN_DIM), dtype=np.float16)
    g[:n] = emb[perm].astype(np.float16)
    g = g.reshape(NPr, 2, 128, IN_DIM).transpose(0, 2, 1, 3)
    return np.ascontiguousarray(g.reshape(NPr, 128, 2 * IN_DIM))


def kernel(nodes_embeddings, rels_embeddings, nodes_ids, rels_ids,
           entity_memory, rel_memory, W_node, b_node, W_rel, b_rel, time):
    nodes_embeddings = np.ascontiguousarray(np.asarray(nodes_embeddings, dtype=np.float32))
    rels_embeddings = np.ascontiguousarray(np.asarray(rels_embeddings, dtype=np.float32))
    nodes_ids = np.asarray(nodes_ids).astype(np.int64)
    rels_ids = np.asarray(rels_ids).astype(np.int64)
    entity_memory = np.asarray(entity_memory, dtype=np.float32)
    rel_memory = np.asarray(rel_memory, dtype=np.float32)
    W_node = np.asarray(W_node, dtype=np.float32)
    b_node = np.asarray(b_node, dtype=np.float32)
    W_rel = np.asarray(W_rel, dtype=np.float32)
    b_rel = np.asarray(b_rel, dtype=np.float32)
    t = float(np.asarray(time))

    inv = np.float32(1.0 / (t + 1.0))
    scale = np.float32(t / (t + 1.0)) if t > 1 else np.float32(1.0)
    has_bias = bool(np.any(b_node != 0.0))

    # ---- host routing ----
    perms_n, NCn = _route(nodes_ids, NSHARD)
    NCn += NCn % 2  # even chunk count for pairing
    perms_r, NCr = _route(rels_ids, RSHARD)

    # sort node events by local row id
    loc_n = []
    for c in range(NCORES):
        loc = nodes_ids[perms_n[c]] - c * NSHARD
        order = np.argsort(loc, kind="stable")
        perms_n[c] = perms_n[c][order]
        loc_n.append(loc[order])

    # union pair spans across cores (shared module shape)
    spans_u = [set() for _ in range(NCn // 2)]
    for c in range(NCORES):
        ps = _pair_spans(_chunk_groups(loc_n[c], NCn))
        for j, s in enumerate(ps):
            spans_u[j].update(s)
    pair_spans = tuple(tuple(sorted(s)) for s in spans_u)

    key = (NCn, NCr, pair_spans, has_bias)
    if key not in _module_cache:
        _module_cache[key] = _build_module(NCn, NCr, pair_spans, has_bias)
    nc = _module_cache[key]

    # ---- host packing ----
    # W_node DoubleRow layout: [1024,512] -> [128p, KP, 2, 512] fp8
    wn = (W_node.T * (inv * W_SCALE)).astype(NP_F8)
    wn = wn.reshape(KP, 2, 128, MEM_DIM).transpose(2, 0, 1, 3)
    wn = np.ascontiguousarray(wn.reshape(128, KP * 2 * MEM_DIM))
    wr = (W_rel.T * inv).astype(np.float16)
    wr = wr.reshape(KT, 128, MEM_DIM).transpose(1, 0, 2)
    wr = np.ascontiguousarray(wr.reshape(128, KT * MEM_DIM))
    bn = np.broadcast_to(b_node * (inv * W_SCALE), (128, MEM_DIM)).astype(np.float32).copy()
    ns_max = max(((s[-1] - s[0] + 1) for s in pair_spans if s), default=1)
    iw = max(ns_max * 128, 128)
    iota = np.broadcast_to(np.arange(iw, dtype=np.float32), (128, iw)).copy()
    ident = np.eye(128, dtype=np.float16)

    in_maps = []
    for c in range(NCORES):
        lo_n, hi_n = c * NSHARD, min((c + 1) * NSHARD, N_NODES)
        lo_r, hi_r = c * RSHARD, min((c + 1) * RSHARD, N_RELS)
        # memory shard: scaled, bf16, transposed to [128, 98*512]
        mem_shard = np.zeros((NSHARD, MEM_DIM), dtype=np.float32)
        mem_shard[:hi_n - lo_n] = entity_memory[lo_n:hi_n]
        mem_shard *= scale
        mem_tp = np.ascontiguousarray(
            mem_shard.astype(MEM_NP).reshape(NGROUPS, 128, MEM_DIM)
            .transpose(1, 0, 2).reshape(128, NGROUPS * MEM_DIM))
        # rel memory shard: scale + count-weighted bias folded on host
        rmem_shard = np.zeros((RSHARD, MEM_DIM), dtype=np.float32)
        rmem_shard[:hi_r - lo_r] = rel_memory[lo_r:hi_r] * scale
        loc_r = rels_ids[perms_r[c]] - lo_r
        cnt = np.bincount(loc_r, minlength=RSHARD).astype(np.float32)
        rmem_shard += np.outer(cnt, b_rel * inv)
        in_maps.append(dict(
            emb_n=_pack_emb_n(nodes_embeddings, perms_n[c], NCn),
            emb_r=_pack_emb_r(rels_embeddings, perms_r[c], NCr),
            ids_n=_pack_ids(loc_n[c], NCn),
            ids_r=_pack_ids(loc_r, NCr),
            w_n=wn, w_r=wr, b_n=bn, iota_in=iota, ident_in=ident,
            rmem_s=rmem_shard, mem_t=mem_tp,
        ))

    trace = bool(int(os.environ.get("KERNEL_TRACE", "0"))) and _ensure_ntff_hook()
    try:
        res = run_bass_kernel_spmd(
            nc, in_maps, core_ids=list(range(NCORES)),
            trace=trace, trace_cores=list(range(NCORES)) if trace else None)
    except Exception:
        res = run_bass_kernel_spmd(
            nc, in_maps, core_ids=list(range(NCORES)),
            trace=trace, trace_cores=list(range(NCORES)) if trace else None)
    kernel.last_exec_time_ns = res.exec_time_ns
    kernel.last_results = res

    out = np.empty((N_NODES + N_RELS, MEM_DIM), dtype=np.float32)
    for c in range(NCORES):
        lo_n, hi_n = c * NSHARD, min((c + 1) * NSHARD, N_NODES)
        on = np.asarray(res.results[c]["out_n"]).reshape(128, NGROUPS, MEM_DIM)
        on = on.transpose(1, 0, 2).reshape(NSHARD, MEM_DIM).astype(np.float32)
        out[lo_n:hi_n] = on[:hi_n - lo_n]
        lo_r, hi_r = c * RSHARD, min((c + 1) * RSHARD, N_RELS)
        out[N_NODES + lo_r:N_NODES + hi_r] = \
            np.asarray(res.results[c]["out_r"])[:hi_r - lo_r]
    return out


# revision 20
# speedup vs baseline: 1.0682x; 1.0682x over previous
"""Trainium2 Bass kernel for nn_Memory scatter_memory problem — v2.

Reference computation:
    scale = t/(t+1) if t > 1 else 1 ; inv = 1/(t+1)
    entity_memory = entity_memory*scale ; .at[nodes_ids].add((nodes_emb @ W_node.T + b_node)*inv)
    rel_memory    = rel_memory*scale    ; .at[rels_ids].add((rels_emb @ W_rel.T + b_rel)*inv)
    out = concat([entity_memory, rel_memory])   # [100500, 512]

v2 strategy (8 NeuronCores, SPMD):
  - Row-shard both tables; host routes events to owner core, sorts node
    events by local row id (baseline approach).
  - NODES: fp8(e4m3) DoubleRow projection (2 k-tiles per matmul, half
    cycles/col), updates quantized to fp8, scatter-add via fp8 DoubleRow
    one-hot matmuls over chunk PAIRS (2 chunks per scatter matmul).
  - RELS: aggregate raw embeddings by rel id first (one-hot matmul over
    events, fp16), then ONE tiny projection of the 64 aggregated rows.
    Removes the entire per-event rel projection.
  - Memory table streamed bf16 in, out written bf16 (nodes) / f32 (rels).
  - Elementwise spread across DVE / Pool / Act engines; batched DMAs.
"""

import os
import sys
import numpy as np

for _p in ("/root/.axon_site", "/root/.axon_site/_ro/trn_rl_repo",
           "/root/.axon_site/_ro/pypackages", "/opt/trn_rl_repo"):
    if os.path.isdir(_p) and _p not in sys.path:
        sys.path.append(_p)

import ml_dtypes
import concourse.bacc as bacc
import concourse.mybir as mybir
import concourse.tile as tile
from concourse.bass_utils import run_bass_kernel_spmd

F32 = mybir.dt.float32
F16 = mybir.dt.float16
BF16 = mybir.dt.bfloat16
F8 = mybir.dt.float8e4
AL = mybir.AluOpType
DR = mybir.MatmulPerfMode.DoubleRow

NP_F8 = ml_dtypes.float8_e4m3
NP_BF16 = ml_dtypes.bfloat16

N_NODES = 100000
N_RELS = 500
MEM_DIM = 512
IN_DIM = 1024
NCORES = 8
NSHARD = 12544          # 98 * 128 rows per core (core 7 ragged, padded)
NGROUPS = NSHARD // 128  # 98
RSHARD = 64             # rel rows per core (core 7 ragged, padded)
KT = IN_DIM // 128      # 8 k-tiles
KP = KT // 2            # 4 DoubleRow k-pairs
PAD_ID = 1.0e6
W_SCALE = 128.0         # fp8 scaling of W_node*inv (descale at merge)
GB = 8                  # groups per mem/out DMA batch
NB = (NGROUPS + GB - 1) // GB
MEM_BF16 = bool(int(os.environ.get("KERNEL_MEM_BF16", "0")))
MEM_DT = BF16 if MEM_BF16 else F8
MEM_NP = NP_BF16 if MEM_BF16 else NP_F8

_module_cache = {}


def _ensure_ntff_hook():
    """Register the axon NTFF profile hook (missing antenv.axon_hooks shim)."""
    import types
    try:
        from antenv.axon_hooks import get_axon_ntff_profile_hook
        return get_axon_ntff_profile_hook() is not None
    except ImportError:
        pass
    try:
        import antenv
        from trn_agent_boot.trn_boot import _ntff_profile_via_ctypes
        import concourse.bass_utils as bu
        mod = types.ModuleType("antenv.axon_hooks")
        state = {"h": None}
        mod.set_axon_ntff_profile_hook = lambda h: state.__setitem__("h", h)
        mod.get_axon_ntff_profile_hook = lambda: state["h"]
        sys.modules["antenv.axon_hooks"] = mod
        antenv.axon_hooks = mod
        h = _ntff_profile_via_ctypes("/opt/axon/libaxon_pjrt.so")
        mod.set_axon_ntff_profile_hook(h)
        bu.upload_artifacts = lambda tmpdir: f"local:{tmpdir}"
        return h is not None
    except Exception:
        return False


def _build_module(NCn, NCr, pair_spans, has_bias):
    """Build the SPMD Bacc module.

    NCn: node event chunks (even; last may be all-pad). NCr: rel chunks.
    pair_spans: tuple over pair j of tuple of groups touched by chunk 2j
    or 2j+1 (union, sorted).
    """
    nc = bacc.Bacc(None, target_bir_lowering=False)

    NPn = NCn // 2
    NPr = (NCr + 1) // 2
    # widest contiguous group range any pair spans (for one-hot tiles)
    NS_MAX = max((s[-1] - s[0] + 1) for s in pair_spans if s)
    IW = max(NS_MAX * 128, 128)

    emb_n = nc.dram_tensor("emb_n", [NPn, 128, 2 * KT * 128], F8, kind="ExternalInput")
    emb_r = nc.dram_tensor("emb_r", [NPr, 128, 2 * IN_DIM], F16, kind="ExternalInput")
    ids_n = nc.dram_tensor("ids_n", [128, NCn], F32, kind="ExternalInput")
    ids_r = nc.dram_tensor("ids_r", [128, NCr], F32, kind="ExternalInput")
    w_n = nc.dram_tensor("w_n", [128, KP * 2 * MEM_DIM], F8, kind="ExternalInput")
    w_r = nc.dram_tensor("w_r", [128, KT * MEM_DIM], F16, kind="ExternalInput")
    b_n = nc.dram_tensor("b_n", [128, MEM_DIM], F32, kind="ExternalInput")
    iota_in = nc.dram_tensor("iota_in", [128, IW], F32, kind="ExternalInput")
    ident_in = nc.dram_tensor("ident_in", [128, 128], F16, kind="ExternalInput")
    rmem_s = nc.dram_tensor("rmem_s", [RSHARD, MEM_DIM], F32, kind="ExternalInput")
    mem_t = nc.dram_tensor("mem_t", [128, NGROUPS * MEM_DIM], MEM_DT, kind="ExternalInput")
    out_n = nc.dram_tensor("out_n", [128, NGROUPS * MEM_DIM], BF16, kind="ExternalOutput")
    out_r = nc.dram_tensor("out_r", [RSHARD, MEM_DIM], F32, kind="ExternalOutput")

    # group lifetime over pairs
    first_pair, last_pair = {}, {}
    for j, gs in enumerate(pair_spans):
        for g in gs:
            first_pair.setdefault(g, j)
            last_pair[g] = j
    # PSUM budget
    maxopen, open_now = 0, set()
    for j, gs in enumerate(pair_spans):
        open_now.update(gs)
        maxopen = max(maxopen, len(open_now))
        for g in list(open_now):
            if last_pair[g] == j:
                open_now.discard(g)
    pg_bufs = min(max(maxopen, 1), 5)
    pu_bufs = 8 - 2 - pg_bufs  # agg pool has 2 banks
    assert pu_bufs >= 1, f"PSUM overflow: maxopen={maxopen}"

    with tile.TileContext(nc) as tc:
        with tc.tile_pool(name="const", bufs=1) as cpool, \
             tc.tile_pool(name="en", bufs=8) as enpool, \
             tc.tile_pool(name="er", bufs=8) as erpool, \
             tc.tile_pool(name="oh", bufs=4) as ohpool, \
             tc.tile_pool(name="ohr", bufs=6) as ohrpool, \
             tc.tile_pool(name="upd", bufs=4) as updpool, \
             tc.tile_pool(name="mem", bufs=4) as mpool, \
             tc.tile_pool(name="outs", bufs=4) as opool, \
             tc.tile_pool(name="tail", bufs=2) as tpool, \
             tc.tile_pool(name="pu", bufs=pu_bufs, space="PSUM") as pupool, \
             tc.tile_pool(name="pg", bufs=pg_bufs, space="PSUM") as pgpool, \
             tc.tile_pool(name="pa", bufs=2, space="PSUM") as papool:

            # ---- startup-critical constants only (tail consts deferred) ----
            t_wn = cpool.tile([128, KP, 2, MEM_DIM], F8, tag="wn")
            nc.sync.dma_start(t_wn[:], w_n.ap().rearrange("p (k i n) -> p k i n", k=KP, i=2))
            t_iota = cpool.tile([128, IW], F32, tag="iota")
            nc.scalar.dma_start(t_iota[:], iota_in[:])
            t_ids_n = cpool.tile([128, NCn], F32, tag="idsn")
            nc.scalar.dma_start(t_ids_n[:], ids_n[:])
            t_ids_r = cpool.tile([128, NCr], F32, tag="idsr")
            nc.scalar.dma_start(t_ids_r[:], ids_r[:])
            if has_bias:
                t_bn = cpool.tile([128, MEM_DIM], F32, tag="bn")
                nc.scalar.dma_start(t_bn[:], b_n[:])
            tail_consts = {}

            def fetch_tail_consts():
                t_wr = cpool.tile([128, KT, MEM_DIM], F16, tag="wr")
                nc.scalar.dma_start(t_wr[:], w_r.ap().rearrange("p (k n) -> p k n", k=KT))
                t_ident = cpool.tile([128, 128], F16, tag="ident")
                nc.scalar.dma_start(t_ident[:], ident_in[:])
                t_rmem = cpool.tile([RSHARD, MEM_DIM], F32, tag="rmem")
                nc.scalar.dma_start(t_rmem[:], rmem_s[:])
                tail_consts.update(wr=t_wr, ident=t_ident, rmem=t_rmem)

            # ---- mem-in batches (prefetched ahead of merge frontier) ----
            mem_tiles = {}

            def fetch_mem(b):
                if b in mem_tiles or b >= NB:
                    return
                gcnt = min(GB, NGROUPS - b * GB)
                t = mpool.tile([128, GB, MEM_DIM], MEM_DT, tag="mem", name=f"mem_{b}")
                nc.gpsimd.dma_start(
                    t[:, :gcnt, :],
                    mem_t.ap()[:, b * GB * MEM_DIM:(b * GB + gcnt) * MEM_DIM]
                    .rearrange("p (g n) -> p g n", g=gcnt))
                mem_tiles[b] = t

            for _b in range(3):
                fetch_mem(_b)

            # ---- rel aggregation PSUM (accumulates across all rel chunks) ----
            p_agg_a = papool.tile([RSHARD, MEM_DIM], F32, tag="pa", name="agg_a")
            p_agg_b = papool.tile([RSHARD, MEM_DIM], F32, tag="pa", name="agg_b")

            er_pair = {}

            def rel_chunk(ec):
                if ec % 2 == 0:
                    t_pe = erpool.tile([128, 2, IN_DIM], F16, tag="er", name=f"er_{ec}")
                    nc.sync.dma_start(
                        t_pe[:], emb_r[ec // 2].rearrange("p (c x) -> p c x", c=2))
                    er_pair[ec // 2] = t_pe
                t_er = er_pair[ec // 2][:, ec % 2]
                t_ohr = ohrpool.tile([128, RSHARD], F16, tag="ohr", name=f"ohr_{ec}")
                nc.vector.tensor_scalar(
                    t_ohr[:], t_iota[:, :RSHARD], 0.0, t_ids_r[:, ec:ec + 1],
                    op0=AL.add, op1=AL.is_equal)
                st = (ec == 0)
                sp = (ec == NCr - 1)
                nc.tensor.matmul(p_agg_a[:], t_ohr[:], t_er[:, 0:MEM_DIM],
                                 start=st, stop=sp, skip_group_check=True)
                nc.tensor.matmul(p_agg_b[:], t_ohr[:], t_er[:, MEM_DIM:IN_DIM],
                                 start=st, stop=sp, skip_group_check=True)

            # ---- node pair machinery ----
            en_pair = {}
            pair_state = {}
            grp_psum = {}

            def node_pair_front(j):
                """DMA + proj + upd-quantize + one-hots for pair j."""
                t_pp = enpool.tile([128, 2, KP, 2, 128], F8, tag="en", name=f"en_{j}")
                nc.sync.dma_start(
                    t_pp[:], emb_n[j].rearrange("p (c k i e) -> p c k i e",
                                                c=2, k=KP, i=2))
                en_pair[j] = t_pp
                t_up = updpool.tile([128, 2, MEM_DIM], F8, tag="upd", name=f"upd_{j}")
                for i in (0, 1):
                    p_u = pupool.tile([128, MEM_DIM], F32, tag="pu", name=f"pu_{j}_{i}")
                    for k in range(KP):
                        nc.tensor.matmul(p_u[:], t_pp[:, i, k], t_wn[:, k],
                                         start=(k == 0), stop=(k == KP - 1),
                                         perf_mode=DR)
                    if has_bias:
                        nc.vector.scalar_tensor_tensor(
                            t_up[:, i, :], p_u[:], 1.0, t_bn[:],
                            op0=AL.mult, op1=AL.add)
                    else:
                        nc.scalar.activation(
                            t_up[:, i, :], p_u[:],
                            mybir.ActivationFunctionType.Copy)
                # one wide is_equal per chunk half covering the pair's whole
                # contiguous group range; scatter matmuls slice per group
                gs = pair_spans[j]
                t_oh = None
                if gs:
                    g0 = gs[0]
                    ns = gs[-1] - g0 + 1
                    t_oh = ohpool.tile([128, 2, NS_MAX * 128], F8, tag="oh",
                                       name=f"oh_{j}")
                    for i in (0, 1):
                        nc.vector.tensor_scalar(
                            t_oh[:, i, :ns * 128], t_iota[:, :ns * 128],
                            float(g0 * 128),
                            t_ids_n[:, 2 * j + i:2 * j + i + 1],
                            op0=AL.add, op1=AL.is_equal)
                pair_state[j] = (t_up, t_oh)

            def node_pair_scatter(j):
                t_up, t_oh = pair_state.pop(j)
                gs = pair_spans[j]
                if not gs:
                    return
                g0 = gs[0]
                for g in gs:
                    if g not in grp_psum:
                        grp_psum[g] = pgpool.tile([128, MEM_DIM], F32, tag="pg",
                                                  name=f"pg_{g}")
                    off = (g - g0) * 128
                    nc.tensor.matmul(grp_psum[g][:],
                                     t_oh[:, :, off:off + 128], t_up[:],
                                     start=(first_pair[g] == j),
                                     stop=(last_pair[g] == j),
                                     perf_mode=DR, skip_group_check=True)

            # ---- merge frontier (strict ascending group order) ----
            out_state = {"b": -1, "tile": None}
            frontier = {"g": 0}

            def flush_out():
                b = out_state["b"]
                if b < 0:
                    return
                gcnt = min(GB, NGROUPS - b * GB)
                nc.gpsimd.dma_start(
                    out_n.ap()[:, b * GB * MEM_DIM:(b * GB + gcnt) * MEM_DIM]
                    .rearrange("p (g n) -> p g n", g=gcnt),
                    out_state["tile"][:, :gcnt, :])
                out_state["b"] = -1
                out_state["tile"] = None

            def merge_group(g):
                b, s = g // GB, g % GB
                if out_state["b"] != b:
                    flush_out()
                    out_state["b"] = b
                    out_state["tile"] = opool.tile([128, GB, MEM_DIM], BF16,
                                                   tag="outs", name=f"outs_{b}")
                    fetch_mem(b + 3)
                t_mem = mem_tiles[b]
                t_out = out_state["tile"]
                if g in grp_psum:
                    eng = nc.vector
                    eng.scalar_tensor_tensor(
                        t_out[:, s, :], grp_psum[g][:], 1.0 / W_SCALE,
                        t_mem[:, s, :], op0=AL.mult, op1=AL.add)
                    del grp_psum[g]
                else:
                    nc.scalar.activation(t_out[:, s, :], t_mem[:, s, :],
                                         mybir.ActivationFunctionType.Copy)

            def merge_upto(jdone):
                g = frontier["g"]
                while g < NGROUPS and last_pair.get(g, -1) <= jdone:
                    merge_group(g)
                    g += 1
                frontier["g"] = g

            # ---- rel tail: copy agg out, transpose, project, merge ----
            # (emitted right after the last rel chunk so it overlaps the
            # remaining node pairs)
            def rel_tail():
                t_wr, t_ident, t_rmem = (tail_consts["wr"], tail_consts["ident"],
                                         tail_consts["rmem"])
                t_agg = tpool.tile([RSHARD, IN_DIM], F16, tag="agg")
                nc.scalar.activation(t_agg[:, 0:MEM_DIM], p_agg_a[:],
                                     mybir.ActivationFunctionType.Copy)
                nc.scalar.activation(t_agg[:, MEM_DIM:IN_DIM], p_agg_b[:],
                                     mybir.ActivationFunctionType.Copy)
                p_tr = papool.tile([128, KT, RSHARD], F16, tag="pa", name="p_tr")
                for k in range(KT):
                    nc.tensor.transpose(p_tr[:, k, :],
                                        t_agg[:, k * 128:(k + 1) * 128],
                                        t_ident[:RSHARD, :RSHARD])
                t_aggT = tpool.tile([128, KT, RSHARD], F16, tag="aggT")
                nc.scalar.activation(t_aggT[:], p_tr[:],
                                     mybir.ActivationFunctionType.Copy)
                p_rel = papool.tile([RSHARD, MEM_DIM], F32, tag="pa", name="p_rel")
                for k in range(KT):
                    nc.tensor.matmul(p_rel[:], t_aggT[:, k, :], t_wr[:, k, :],
                                     start=(k == 0), stop=(k == KT - 1))
                t_outr = tpool.tile([RSHARD, MEM_DIM], F32, tag="outr")
                nc.vector.tensor_tensor(t_outr[:], p_rel[:], t_rmem[:], op=AL.add)
                nc.sync.dma_start(out_r[:], t_outr[:])

            # ---- main pipeline (rels paced to finish slightly early) ----
            LAG = 1
            next_r = 0
            pace = NCr / max(NPn - 4, 1)
            tc_at = max(NPn - 10, 0)
            for j in range(NPn):
                if j == tc_at:
                    fetch_tail_consts()
                node_pair_front(j)
                while next_r < NCr and next_r < (j + 1) * pace:
                    rel_chunk(next_r)
                    next_r += 1
                    if next_r == NCr:
                        rel_tail()
                if j >= LAG:
                    node_pair_scatter(j - LAG)
                    merge_upto(j - LAG)
            for j in range(max(NPn - LAG, 0), NPn):
                node_pair_scatter(j)
                merge_upto(j)
            while next_r < NCr:
                rel_chunk(next_r)
                next_r += 1
                if next_r == NCr:
                    rel_tail()
            merge_upto(NPn + 1)
            frontier_g = frontier["g"]
            for g in range(frontier_g, NGROUPS):
                merge_group(g)
            flush_out()

    nc.finalize()
    return nc


# ---------------- host-side routing / packing ----------------

def _route(ids, n_rows_per_core):
    owner = np.minimum(ids // n_rows_per_core, NCORES - 1)
    perms = []
    for c in range(NCORES):
        ev = np.nonzero(owner == c)[0]
        perms.append(ev)
    nmax = max(len(p) for p in perms)
    NC = (nmax + 127) // 128
    return perms, max(NC, 1)


def _pack_ids(local_ids, NC):
    n = len(local_ids)
    out = np.full(NC * 128, PAD_ID, dtype=np.float32)
    out[:n] = local_ids.astype(np.float32)
    return np.ascontiguousarray(out.reshape(NC, 128).T)  # [128, NC]


def _chunk_groups(loc_sorted, NC):
    """Exact per-chunk touched-group sets."""
    spans = []
    for ec in range(NC):
        seg = loc_sorted[ec * 128:(ec + 1) * 128]
        spans.append(sorted(set(int(v) // 128 for v in seg)))
    return spans


def _pair_spans(chunk_spans):
    NP_ = len(chunk_spans) // 2
    out = []
    for j in range(NP_):
        s = sorted(set(chunk_spans[2 * j]) | set(chunk_spans[2 * j + 1]))
        out.append(tuple(s))
    return tuple(out)


def _pack_emb_n(emb, perm, NCn):
    """emb [B, IN_DIM] f32 -> [NPn, 128, 2*KT*128] fp8 DoubleRow layout."""
    n = len(perm)
    C = NCn * 128
    g = np.zeros((C, IN_DIM), dtype=NP_F8)
    g[:n] = emb[perm].astype(NP_F8)
    # [NCn,128e,KP,2,128p] -> [NCn,128p,KP,2,128e]
    g = g.reshape(NCn, 128, KP, 2, 128).transpose(0, 4, 2, 3, 1)
    # pair chunks: [NPn,2,128,KP,2,128] -> [NPn,128,2,KP,2,128]
    g = g.reshape(NCn // 2, 2, 128, KP, 2, 128).transpose(0, 2, 1, 3, 4, 5)
    return np.ascontiguousarray(g.reshape(NCn // 2, 128, 2 * KT * 128))


def _pack_emb_r(emb, perm, NCr):
    """emb [B, IN_DIM] f32 -> [NPr, 128, 2*IN_DIM] fp16 event-major pairs."""
    n = len(perm)
    NPr = (NCr + 1) // 2
    C = NPr * 256
    g = np.zeros((C, IN_DIM), dtype=np.float16)
    g[:n] = emb[perm].astype(np.float16)
    g = g.reshape(NPr, 2, 128, IN_DIM).transpose(0, 2, 1, 3)
    return np.ascontiguousarray(g.reshape(NPr, 128, 2 * IN_DIM))


def kernel(nodes_embeddings, rels_embeddings, nodes_ids, rels_ids,
           entity_memory, rel_memory, W_node, b_node, W_rel, b_rel, time):
    nodes_embeddings = np.ascontiguousarray(np.asarray(nodes_embeddings, dtype=np.float32))
    rels_embeddings = np.ascontiguousarray(np.asarray(rels_embeddings, dtype=np.float32))
    nodes_ids = np.asarray(nodes_ids).astype(np.int64)
    rels_ids = np.asarray(rels_ids).astype(np.int64)
    entity_memory = np.asarray(entity_memory, dtype=np.float32)
    rel_memory = np.asarray(rel_memory, dtype=np.float32)
    W_node = np.asarray(W_node, dtype=np.float32)
    b_node = np.asarray(b_node, dtype=np.float32)
    W_rel = np.asarray(W_rel, dtype=np.float32)
    b_rel = np.asarray(b_rel, dtype=np.float32)
    t = float(np.asarray(time))

    inv = np.float32(1.0 / (t + 1.0))
    scale = np.float32(t / (t + 1.0)) if t > 1 else np.float32(1.0)
    has_bias = bool(np.any(b_node != 0.0))

    # ---- host routing ----
    perms_n, NCn = _route(nodes_ids, NSHARD)
    NCn += NCn % 2  # even chunk count for pairing
    perms_r, NCr = _route(rels_ids, RSHARD)

    # sort node events by local row id
    loc_n = []
    for c in range(NCORES):
        loc = nodes_ids[perms_n[c]] - c * NSHARD
        order = np.argsort(loc, kind="stable")
        perms_n[c] = perms_n[c][order]
        loc_n.append(loc[order])

    # union pair spans across cores (shared module shape)
    spans_u = [set() for _ in range(NCn // 2)]
    for c in range(NCORES):
        ps = _pair_spans(_chunk_groups(loc_n[c], NCn))
        for j, s in enumerate(ps):
            spans_u[j].update(s)
    pair_spans = tuple(tuple(sorted(s)) for s in spans_u)

    key = (NCn, NCr, pair_spans, has_bias)
    if key not in _module_cache:
        _module_cache[key] = _build_module(NCn, NCr, pair_spans, has_bias)
    nc = _module_cache[key]

    # ---- host packing ----
    # W_node DoubleRow layout: [1024,512] -> [128p, KP, 2, 512] fp8
    wn = (W_node.T * (inv * W_SCALE)).astype(NP_F8)
    wn = wn.reshape(KP, 2, 128, MEM_DIM).transpose(2, 0, 1, 3)
    wn = np.ascontiguousarray(wn.reshape(128, KP * 2 * MEM_DIM))
    wr = (W_rel.T * inv).astype(np.float16)
    wr = wr.reshape(KT, 128, MEM_DIM).transpose(1, 0, 2)
    wr = np.ascontiguousarray(wr.reshape(128, KT * MEM_DIM))
    bn = np.broadcast_to(b_node * (inv * W_SCALE), (128, MEM_DIM)).astype(np.float32).copy()
    ns_max = max(((s[-1] - s[0] + 1) for s in pair_spans if s), default=1)
    iw = max(ns_max * 128, 128)
    iota = np.broadcast_to(np.arange(iw, dtype=np.float32), (128, iw)).copy()
    ident = np.eye(128, dtype=np.float16)

    in_maps = []
    for c in range(NCORES):
        lo_n, hi_n = c * NSHARD, min((c + 1) * NSHARD, N_NODES)
        lo_r, hi_r = c * RSHARD, min((c + 1) * RSHARD, N_RELS)
        # memory shard: scaled, bf16, transposed to [128, 98*512]
        mem_shard = np.zeros((NSHARD, MEM_DIM), dtype=np.float32)
        mem_shard[:hi_n - lo_n] = entity_memory[lo_n:hi_n]
        mem_shard *= scale
        mem_tp = np.ascontiguousarray(
            mem_shard.astype(MEM_NP).reshape(NGROUPS, 128, MEM_DIM)
            .transpose(1, 0, 2).reshape(128, NGROUPS * MEM_DIM))
        # rel memory shard: scale + count-weighted bias folded on host
        rmem_shard = np.zeros((RSHARD, MEM_DIM), dtype=np.float32)
        rmem_shard[:hi_r - lo_r] = rel_memory[lo_r:hi_r] * scale
        loc_r = rels_ids[perms_r[c]] - lo_r
        cnt = np.bincount(loc_r, minlength=RSHARD).astype(np.float32)
        rmem_shard += np.outer(cnt, b_rel * inv)
        in_maps.append(dict(
            emb_n=_pack_emb_n(nodes_embeddings, perms_n[c], NCn),
            emb_r=_pack_emb_r(rels_embeddings, perms_r[c], NCr),
            ids_n=_pack_ids(loc_n[c], NCn),
            ids_r=_pack_ids(loc_r, NCr),
            w_n=wn, w_r=wr, b_n=bn, iota_in=iota, ident_in=ident,
            rmem_s=rmem_shard, mem_t=mem_tp,
        ))

    trace = bool(int(os.environ.get("KERNEL_TRACE", "0"))) and _ensure_ntff_hook()
    try:
        res = run_bass_kernel_spmd(
            nc, in_maps, core_ids=list(range(NCORES)),
            trace=trace, trace_cores=list(range(NCORES)) if trace else None)
    except Exception:
        res = run_bass_kernel_spmd(
            nc, in_maps, core_ids=list(range(NCORES)),
            trace=trace, trace_cores=list(range(NCORES)) if trace else None)
    kernel.last_exec_time_ns = res.exec_time_ns
    kernel.last_results = res

    out = np.empty((N_NODES + N_RELS, MEM_DIM), dtype=np.float32)
    for c in range(NCORES):
        lo_n, hi_n = c * NSHARD, min((c + 1) * NSHARD, N_NODES)
        on = np.asarray(res.results[c]["out_n"]).reshape(128, NGROUPS, MEM_DIM)
        on = on.transpose(1, 0, 2).reshape(NSHARD, MEM_DIM).astype(np.float32)
        out[lo_n:hi_n] = on[:hi_n - lo_n]
        lo_r, hi_r = c * RSHARD, min((c + 1) * RSHARD, N_RELS)
        out[N_NODES + lo_r:N_NODES + hi_r] = \
            np.asarray(res.results[c]["out_r"])[:hi_r - lo_r]
    return out
